# revision 13
# baseline (speedup 1.0000x reference)
"""Grouped sparse-attention Trainium2 kernel (8 NeuronCores, SPMD).

Strategy:
- Host: stable-argsort tokens by category per batch (index math only),
  build per-core group index tables.
- Device (per core, 64 groups of 128 tokens): indirect-DMA row gather of
  qkv, per-group multi-head attention (f32 QK^T with PSUM-bank-safe
  row-tiled matmuls, single-pass ACT exp softmax with matmul-fused max
  subtraction, bf16 P/V), projection, indirect-DMA row scatter of the
  output back to original token order.
- Host: sum the two half-batch output buffers per batch (disjoint rows).
"""
import sys

sys.path.insert(0, "/opt/trn_rl_repo")

import numpy as np

import concourse.bass as bass
import concourse.mybir as mybir
from concourse.bass_utils import run_bass_kernel_spmd
from concourse.masks import make_identity
from concourse.tile import TileContext

B, N, C = 4, 16384, 192
NH, D = 6, 32
GS = 128
NG_CORE = 64          # groups per core
N_CORES = 8
LN100 = float(np.log(100.0))
PERM = [0, 3, 1, 4, 2, 5]  # S-bank layout head order (bank=h%3, sub=h//3)

f32 = mybir.dt.float32
bf16 = mybir.dt.bfloat16
fp16 = mybir.dt.float16


# ---------------------------------------------------------------------------
# Workaround: this walrus build allows only ONE sync-wait per instruction.
# TileContext's tail drain accumulates one wait per pending semaphore; split
# them across multiple single-wait drain instructions.
def _patch_tile():
    from concourse.tile import TileContext as TC

    def _patched(self, tick_clock, wait_clock):
        from concourse.tile import ScopedClock

        drain_inst = self.nc.sync.drain()
        wait_clock.add_sem_waits(
            drain_inst.ins, ScopedClock({None: tick_clock.global_clock})
        )
        si = drain_inst.ins.sync_info
        if si is not None and si.on_wait is not None and len(si.on_wait) > 1:
            waits = list(si.on_wait)
            si.on_wait = [waits[0]]
            for w in waits[1:]:
                d2 = self.nc.sync.drain()
                si2 = d2.ins.sync_info
                if si2 is None:
                    d2.ins.sync_info = mybir.SyncInfo(on_wait=[w], on_update=[])
                else:
                    si2.on_wait = [w]
        self.nc.all_engine_barrier()
        assert self.sems is not None
        popped = self.nc._tile_sem_poison_stack.pop()
        assert popped is self._sem_poison
        self.nc.clear_and_free_semaphores(list(self.sems.allocated().values()))
        self.nc.all_engine_barrier()

    TC._drain_and_barrier = _patched


def _fixup_multiwait(nc):
    """Move extra sync-waits onto standalone event-semaphore instructions."""
    for f in nc.m.functions:
        for bb in f.blocks:
            il = bb.instructions
            i = 0
            while i < len(il):
                inst = il[i]
                si = getattr(inst, "sync_info", None)
                ow = getattr(si, "on_wait", None) if si is not None else None
                if ow is not None and len(ow) > 1:
                    waits = list(ow)
                    si.on_wait = [waits[-1]]
                    for j, w in enumerate(waits[:-1]):
                        ev = mybir.InstEventSemaphore(
                            name=f"{inst.name}_xwait{j}", ins=[], outs=[]
                        )
                        ev.engine = inst.engine
                        ev.sync_info = mybir.SyncInfo(on_wait=[w], on_update=[])
                        nc.register_instruction(ev, overwrite=True)
                        il.insert(i, ev)
                        i += 1
                i += 1


def _install_ntff_hook():
    import types

    if "antenv.axon_hooks" in sys.modules:
        return
    try:
        import antenv
    except ImportError:
        return
    mod = types.ModuleType("antenv.axon_hooks")
    mod._hook = None
    mod.set_axon_ntff_profile_hook = lambda h: setattr(mod, "_hook", h)
    mod.get_axon_ntff_profile_hook = lambda: mod._hook
    sys.modules["antenv.axon_hooks"] = mod
    antenv.axon_hooks = mod
    try:
        from trn_agent_boot.trn_boot import _ntff_profile_via_ctypes

        mod.set_axon_ntff_profile_hook(
            _ntff_profile_via_ctypes("/opt/axon/libaxon_pjrt.so")
        )
    except Exception:
        pass


def build_nc(gat_bufs=12, sb_bufs=6):
    nc = bass.Bass()
    qkv_d = nc.declare_dram_parameter("qkvb", [N, 3 * C], f32, isOutput=False)
    ordv_d = nc.declare_dram_parameter("ordv", [128, NG_CORE], mybir.dt.int32, isOutput=False)
    wt_d = nc.declare_dram_parameter("wt", [C + 1, C], f32, isOutput=False)
    lsc_d = nc.declare_dram_parameter("lsc", [128, 1], f32, isOutput=False)
    out_d = nc.declare_dram_parameter("out", [N, C], f32, isOutput=True)

    with TileContext(nc) as tc:
        with (
            tc.tile_pool(name="cst", bufs=1) as cst,
            tc.tile_pool(name="gp", bufs=gat_bufs) as gp,
            tc.tile_pool(name="sb", bufs=sb_bufs) as sb,
            tc.tile_pool(name="ps_s", bufs=1, space="PSUM") as ps_s,
            tc.tile_pool(name="ps_pt", bufs=2, space="PSUM") as ps_pt,
            tc.tile_pool(name="ps_a", bufs=1, space="PSUM") as ps_a,
            tc.tile_pool(name="ps_b", bufs=2, space="PSUM") as ps_b,
        ):
            # ---- constants ----
            ident = cst.tile([128, 128], f32)
            make_identity(nc, ident[:])
            ident_16 = cst.tile([128, 128], fp16)
            nc.vector.tensor_copy(out=ident_16[:], in_=ident[:])
            wt0 = cst.tile([96, C], fp16)
            wt1 = cst.tile([97, C], fp16)
            nc.gpsimd.dma_start(out=wt0[:], in_=wt_d[0:96, :])
            nc.gpsimd.dma_start(out=wt1[:], in_=wt_d[96:193, :])
            idx_all = cst.tile([128, NG_CORE], mybir.dt.int32)
            nc.sync.dma_start(out=idx_all[:], in_=ordv_d[:])
            # scale vector: exp(min(logit_scale, ln 100)) per partition
            lsc_t = cst.tile([128, 1], f32)
            nc.sync.dma_start(out=lsc_t[:], in_=lsc_d[:])
            lmin = cst.tile([128, 1], f32)
            nc.vector.tensor_scalar_min(out=lmin[:], in0=lsc_t[:], scalar1=LN100)
            scl = cst.tile([128, 1], f32)
            nc.scalar.activation(
                out=scl[:], in_=lmin[:], func=mybir.ActivationFunctionType.Exp,
                bias=0.0, scale=1.0,
            )

            def scol(h):
                return 512 * (h % 3) + 128 * (h // 3)

            for g in range(NG_CORE):
                # 1. gather rows
                gat = gp.tile([128, 3 * C], fp16, tag="gat")
                nc.gpsimd.indirect_dma_start(
                    out=gat[:],
                    out_offset=None,
                    in_=qkv_d[:],
                    in_offset=bass.IndirectOffsetOnAxis(
                        ap=idx_all[:, g:g + 1], axis=0
                    ),
                )
                # 2. q,k transposes -> [96, 512] psum -> SBUF (split DVE/ACT)
                qkT_ps = ps_a.tile([96, 512], fp16, tag="qkT_ps")
                for i in range(4):
                    nc.tensor.transpose(
                        out=qkT_ps[:, i * 128:(i + 1) * 128],
                        in_=gat[:, i * 96:(i + 1) * 96],
                        identity=ident_16[:],
                    )
                qkT = sb.tile([96, 512], fp16, tag="qkT")
                nc.vector.tensor_scalar_mul(
                    out=qkT[:, 0:256], in0=qkT_ps[:, 0:256], scalar1=scl[0:96, 0:1]
                )
                nc.scalar.copy(out=qkT[:, 256:512], in_=qkT_ps[:, 256:512])
                # 3. v cast, interleaved with ones cols: block h = [v_h | 1]
                v_bf = sb.tile([128, NH * (D + 1)], fp16, tag="v_bf")
                nc.vector.tensor_copy(
                    out=v_bf[:].rearrange("p (h e) -> p h e", h=NH)[:, :, 0:D],
                    in_=gat[:, 2 * C:3 * C].rearrange("p (h e) -> p h e", h=NH),
                )
                nc.vector.memset(
                    v_bf[:].rearrange("p (h e) -> p h e", h=NH)[:, :, D:D + 1], 1.0
                )

                # 4-7. per-bank: S matmuls -> reduce -> negms -> exp
                def chan_slice(base, c0):
                    blk = c0 // 96
                    return qkT[
                        c0 % 96:(c0 % 96) + 32,
                        (base + blk) * 128:(base + blk) * 128 + 128,
                    ]

                P = sb.tile([128, NH * GS], fp16, tag="P")
                for b in range(3):
                    S_b = ps_s.tile([128, 512], f32, tag=f"S_b{b}")
                    for s in range(2):
                        h = s * 3 + b
                        c0 = h * 32
                        nc.tensor.matmul(
                            out=S_b[:, s * GS:(s + 1) * GS],
                            lhsT=chan_slice(0, c0),
                            rhs=chan_slice(2, c0),
                            start=(s == 0), stop=(s == 1),
                            skip_group_check=True,
                        )
                    negm_b = sb.tile([128, 2], f32, tag=f"negm{b}")
                    nc.vector.tensor_reduce(
                        out=negm_b[:].rearrange("p (x s) -> p x s", x=1),
                        in_=S_b[:].rearrange("p (s j) -> p s j", s=4)[:, None, 0:2, :],
                        axis=mybir.AxisListType.X, op=mybir.AluOpType.max, negate=True,
                    )
                    for s in range(2):
                        c = 2 * b + s
                        nc.scalar.activation(
                            out=P[:, c * GS:(c + 1) * GS],
                            in_=S_b[:, s * GS:s * GS + GS],
                            func=mybir.ActivationFunctionType.Exp,
                            bias=negm_b[:, s:s + 1], scale=1.0,
                        )
                # 8. P^T per head
                PT_ps = ps_pt.tile([128, NH * GS], fp16, tag="PT_ps")
                for c in range(NH):
                    nc.tensor.transpose(
                        out=PT_ps[:, c * GS:(c + 1) * GS],
                        in_=P[:, c * GS:(c + 1) * GS],
                        identity=ident_16[:],
                    )
                PT = sb.tile([128, NH * GS], fp16, tag="PT")
                nc.vector.tensor_copy(out=PT[:, 0:384], in_=PT_ps[:, 0:384])
                nc.scalar.copy(out=PT[:, 384:768], in_=PT_ps[:, 384:768])

                # 9. [O_h | D_h] per head in one matmul (rhs = [v_h | 1])
                OD_ps = ps_b.tile([128, NH * (D + 1)], f32, tag="odr")
                for c in range(NH):
                    h = PERM[c]
                    nc.tensor.matmul(
                        out=OD_ps[:, h * (D + 1):(h + 1) * (D + 1)],
                        lhsT=PT[:, c * GS:(c + 1) * GS],
                        rhs=v_bf[:, h * (D + 1):(h + 1) * (D + 1)],
                        start=(c == 0), stop=(c == NH - 1), skip_group_check=True,
                    )
                # 10. divide (strided D cols at offset 32 of each 33-block)
                recip = sb.tile([128, NH], f32, tag="recip")
                nc.vector.reciprocal(
                    out=recip[:],
                    in_=OD_ps[:].rearrange("p (h e) -> p h e", h=NH)[:, :, D:D + 1],
                )
                Obar = sb.tile([128, C + 2], fp16, tag="Obar")
                nc.vector.tensor_tensor(
                    out=Obar[:, 0:C].rearrange("p (h d) -> p h d", h=NH),
                    in0=OD_ps[:].rearrange("p (h e) -> p h e", h=NH)[:, :, 0:D],
                    in1=recip[:, :, None].to_broadcast([128, NH, D]),
                    op=mybir.AluOpType.mult,
                )
                nc.vector.memset(Obar[:, C:C + 1], 1.0)
                # 11. ObarT
                ObarT_ps = ps_b.tile([97, 256], fp16, tag="odr")
                nc.tensor.transpose(
                    out=ObarT_ps[0:96, 0:128], in_=Obar[:, 0:96], identity=ident_16[:]
                )
                nc.tensor.transpose(
                    out=ObarT_ps[0:97, 128:256], in_=Obar[:, 96:193], identity=ident_16[:]
                )
                ObarT = sb.tile([97, 256], fp16, tag="ObarT")
                nc.vector.tensor_copy(out=ObarT[:], in_=ObarT_ps[:])
                # 12. projection (bias folded into wt1 row 96 / ObarT ones row)
                R_ps = ps_b.tile([128, C], f32, tag="odr")
                nc.tensor.matmul(out=R_ps[:], lhsT=ObarT[0:96, 0:128], rhs=wt0[:],
                                 start=True, stop=False, skip_group_check=True)
                nc.tensor.matmul(out=R_ps[:], lhsT=ObarT[0:97, 128:256], rhs=wt1[:],
                                 start=False, stop=True, skip_group_check=True)
                R = gp.tile([128, C], f32, tag="R")
                nc.vector.tensor_copy(out=R[:], in_=R_ps[:])
                # 13. scatter
                nc.gpsimd.indirect_dma_start(
                    out=out_d[:],
                    out_offset=bass.IndirectOffsetOnAxis(
                        ap=idx_all[:, g:g + 1], axis=0
                    ),
                    in_=R[:],
                    in_offset=None,
                )

    _fixup_multiwait(nc)
    return nc


_NC_CACHE = {}


def _get_nc():
    if "nc" not in _NC_CACHE:
        _patch_tile()
        _install_ntff_hook()
        _NC_CACHE["nc"] = build_nc()
    return _NC_CACHE["nc"]


def _host_prep(qkv, indices, proj_w, proj_b, logit_scale):
    qkv = np.asarray(qkv, dtype=np.float32)
    idx = np.asarray(indices)
    proj_w = np.asarray(proj_w, dtype=np.float32)
    proj_b = np.asarray(proj_b, dtype=np.float32)
    lsc = float(np.asarray(logit_scale).reshape(-1)[0])

    tk = idx.reshape(B, N)
    order = np.argsort(tk, axis=-1, kind="stable").astype(np.int32)  # (B, N)

    wt = np.ascontiguousarray(
        np.concatenate([proj_w.T, proj_b[None, :]], axis=0)
    )
    lsc_vec = np.full((128, 1), lsc, np.float32)

    in_maps = []
    for k in range(N_CORES):
        b = k // 2
        half = k % 2
        cols = order[b, half * 8192:(half + 1) * 8192].reshape(NG_CORE, 128).T
        in_maps.append(
            {
                "qkvb": np.ascontiguousarray(qkv[b]),
                "ordv": np.ascontiguousarray(cols),
                "wt": wt,
                "lsc": lsc_vec,
            }
        )
    return in_maps


def kernel(qkv, indices, proj_w, proj_b, logit_scale, _trace=False, _tmpdir=None):
    nc = _get_nc()
    in_maps = _host_prep(qkv, indices, proj_w, proj_b, logit_scale)
    res = run_bass_kernel_spmd(
        nc, in_maps, list(range(N_CORES)), trace=_trace, tmpdir=_tmpdir
    )
    out = np.empty((B, N, C), np.float32)
    for b in range(B):
        out[b] = res.results[2 * b]["out"] + res.results[2 * b + 1]["out"]
    kernel.last_exec_time_ns = res.exec_time_ns
    return out


# revision 14
# speedup vs baseline: 1.0936x; 1.0936x over previous
"""Grouped sparse-attention Trainium2 kernel (8 NeuronCores, SPMD).

Strategy:
- Host: stable-argsort tokens by category per batch (index math only),
  build per-core group index tables.
- Device (per core, 64 groups of 128 tokens): indirect-DMA row gather of
  qkv, per-group multi-head attention (f32 QK^T with PSUM-bank-safe
  row-tiled matmuls, single-pass ACT exp softmax with matmul-fused max
  subtraction, bf16 P/V), projection, indirect-DMA row scatter of the
  output back to original token order.
- Host: sum the two half-batch output buffers per batch (disjoint rows).
"""
import sys

sys.path.insert(0, "/opt/trn_rl_repo")

import numpy as np

import concourse.bass as bass
import concourse.mybir as mybir
from concourse.bass_utils import run_bass_kernel_spmd
from concourse.masks import make_identity
from concourse.tile import TileContext

B, N, C = 4, 16384, 192
NH, D = 6, 32
GS = 128
NG_CORE = 64          # groups per core
N_CORES = 8
LN100 = float(np.log(100.0))
PERM = [0, 3, 1, 4, 2, 5]  # S-bank layout head order (bank=h%3, sub=h//3)

f32 = mybir.dt.float32
bf16 = mybir.dt.bfloat16
fp16 = mybir.dt.float16


# ---------------------------------------------------------------------------
# Workaround: this walrus build allows only ONE sync-wait per instruction.
# TileContext's tail drain accumulates one wait per pending semaphore; split
# them across multiple single-wait drain instructions.
def _patch_tile():
    from concourse.tile import TileContext as TC

    def _patched(self, tick_clock, wait_clock):
        from concourse.tile import ScopedClock

        drain_inst = self.nc.sync.drain()
        wait_clock.add_sem_waits(
            drain_inst.ins, ScopedClock({None: tick_clock.global_clock})
        )
        si = drain_inst.ins.sync_info
        if si is not None and si.on_wait is not None and len(si.on_wait) > 1:
            waits = list(si.on_wait)
            si.on_wait = [waits[0]]
            for w in waits[1:]:
                d2 = self.nc.sync.drain()
                si2 = d2.ins.sync_info
                if si2 is None:
                    d2.ins.sync_info = mybir.SyncInfo(on_wait=[w], on_update=[])
                else:
                    si2.on_wait = [w]
        self.nc.all_engine_barrier()
        assert self.sems is not None
        popped = self.nc._tile_sem_poison_stack.pop()
        assert popped is self._sem_poison
        self.nc.clear_and_free_semaphores(list(self.sems.allocated().values()))
        self.nc.all_engine_barrier()

    TC._drain_and_barrier = _patched


def _fixup_multiwait(nc):
    """Move extra sync-waits onto standalone event-semaphore instructions."""
    for f in nc.m.functions:
        for bb in f.blocks:
            il = bb.instructions
            i = 0
            while i < len(il):
                inst = il[i]
                si = getattr(inst, "sync_info", None)
                ow = getattr(si, "on_wait", None) if si is not None else None
                if ow is not None and len(ow) > 1:
                    waits = list(ow)
                    si.on_wait = [waits[-1]]
                    for j, w in enumerate(waits[:-1]):
                        ev = mybir.InstEventSemaphore(
                            name=f"{inst.name}_xwait{j}", ins=[], outs=[]
                        )
                        ev.engine = inst.engine
                        ev.sync_info = mybir.SyncInfo(on_wait=[w], on_update=[])
                        nc.register_instruction(ev, overwrite=True)
                        il.insert(i, ev)
                        i += 1
                i += 1


def _install_ntff_hook():
    import types

    if "antenv.axon_hooks" in sys.modules:
        return
    try:
        import antenv
    except ImportError:
        return
    mod = types.ModuleType("antenv.axon_hooks")
    mod._hook = None
    mod.set_axon_ntff_profile_hook = lambda h: setattr(mod, "_hook", h)
    mod.get_axon_ntff_profile_hook = lambda: mod._hook
    sys.modules["antenv.axon_hooks"] = mod
    antenv.axon_hooks = mod
    try:
        from trn_agent_boot.trn_boot import _ntff_profile_via_ctypes

        mod.set_axon_ntff_profile_hook(
            _ntff_profile_via_ctypes("/opt/axon/libaxon_pjrt.so")
        )
    except Exception:
        pass


def build_nc(gat_bufs=12, sb_bufs=6):
    nc = bass.Bass()
    qkv_d = nc.declare_dram_parameter("qkvb", [N, 3 * C], f32, isOutput=False)
    ordv_d = nc.declare_dram_parameter("ordv", [128, NG_CORE], mybir.dt.int32, isOutput=False)
    wt_d = nc.declare_dram_parameter("wt", [C + 1, C], f32, isOutput=False)
    lsc_d = nc.declare_dram_parameter("lsc", [128, 1], f32, isOutput=False)
    out_d = nc.declare_dram_parameter("out", [N, C], f32, isOutput=True)

    with TileContext(nc) as tc:
        with (
            tc.tile_pool(name="cst", bufs=1) as cst,
            tc.tile_pool(name="gp", bufs=gat_bufs) as gp,
            tc.tile_pool(name="sb", bufs=sb_bufs) as sb,
            tc.tile_pool(name="ps_s", bufs=1, space="PSUM") as ps_s,
            tc.tile_pool(name="ps_pt", bufs=2, space="PSUM") as ps_pt,
            tc.tile_pool(name="ps_a", bufs=2, space="PSUM") as ps_a,
            tc.tile_pool(name="ps_b", bufs=1, space="PSUM") as ps_b,
        ):
            # ---- constants ----
            ident = cst.tile([128, 128], f32)
            make_identity(nc, ident[:])
            ident_16 = cst.tile([128, 128], fp16)
            nc.vector.tensor_copy(out=ident_16[:], in_=ident[:])
            wt0 = cst.tile([96, C], fp16)
            wt1 = cst.tile([97, C], fp16)
            nc.gpsimd.dma_start(out=wt0[:], in_=wt_d[0:96, :])
            nc.gpsimd.dma_start(out=wt1[:], in_=wt_d[96:193, :])
            idx_all = cst.tile([128, NG_CORE], mybir.dt.int32)
            nc.sync.dma_start(out=idx_all[:], in_=ordv_d[:])
            # scale vector: exp(min(logit_scale, ln 100)) per partition
            lsc_t = cst.tile([128, 1], f32)
            nc.sync.dma_start(out=lsc_t[:], in_=lsc_d[:])
            lmin = cst.tile([128, 1], f32)
            nc.vector.tensor_scalar_min(out=lmin[:], in0=lsc_t[:], scalar1=LN100)
            scl = cst.tile([128, 1], f32)
            nc.scalar.activation(
                out=scl[:], in_=lmin[:], func=mybir.ActivationFunctionType.Exp,
                bias=0.0, scale=1.0,
            )

            def scol(h):
                return 512 * (h % 3) + 128 * (h // 3)

            for g in range(NG_CORE):
                # 1. gather rows
                gat = gp.tile([128, 3 * C], fp16, tag="gat")
                nc.gpsimd.indirect_dma_start(
                    out=gat[:],
                    out_offset=None,
                    in_=qkv_d[:],
                    in_offset=bass.IndirectOffsetOnAxis(
                        ap=idx_all[:, g:g + 1], axis=0
                    ),
                )
                # 2. q,k transposes -> [96, 512] psum -> SBUF (split DVE/ACT)
                qkT_ps = ps_a.tile([96, 512], fp16, tag="qkT_ps")
                for i in range(4):
                    nc.tensor.transpose(
                        out=qkT_ps[:, i * 128:(i + 1) * 128],
                        in_=gat[:, i * 96:(i + 1) * 96],
                        identity=ident_16[:],
                    )
                qkT = sb.tile([96, 512], fp16, tag="qkT")
                nc.vector.tensor_scalar_mul(
                    out=qkT[:, 0:256], in0=qkT_ps[:, 0:256], scalar1=scl[0:96, 0:1]
                )
                nc.scalar.copy(out=qkT[:, 256:512], in_=qkT_ps[:, 256:512])
                # 3. v cast, interleaved with ones cols: block h = [v_h | 1]
                v_bf = sb.tile([128, NH * (D + 1)], fp16, tag="v_bf")
                nc.vector.tensor_copy(
                    out=v_bf[:].rearrange("p (h e) -> p h e", h=NH)[:, :, 0:D],
                    in_=gat[:, 2 * C:3 * C].rearrange("p (h e) -> p h e", h=NH),
                )
                nc.vector.memset(
                    v_bf[:].rearrange("p (h e) -> p h e", h=NH)[:, :, D:D + 1], 1.0
                )

                # 4-7. per-bank: S matmuls -> reduce -> negms -> exp
                def chan_slice(base, c0):
                    blk = c0 // 96
                    return qkT[
                        c0 % 96:(c0 % 96) + 32,
                        (base + blk) * 128:(base + blk) * 128 + 128,
                    ]

                P = sb.tile([128, NH * GS], fp16, tag="P")
                for b in range(3):
                    S_b = ps_s.tile([128, 512], f32, tag=f"S_b{b}")
                    for s in range(2):
                        h = s * 3 + b
                        c0 = h * 32
                        nc.tensor.matmul(
                            out=S_b[:, s * GS:(s + 1) * GS],
                            lhsT=chan_slice(0, c0),
                            rhs=chan_slice(2, c0),
                            start=(s == 0), stop=(s == 1),
                            skip_group_check=True,
                        )
                    negm_b = sb.tile([128, 2], f32, tag=f"negm{b}")
                    nc.vector.tensor_reduce(
                        out=negm_b[:].rearrange("p (x s) -> p x s", x=1),
                        in_=S_b[:].rearrange("p (s j) -> p s j", s=4)[:, None, 0:2, :],
                        axis=mybir.AxisListType.X, op=mybir.AluOpType.max, negate=True,
                    )
                    for s in range(2):
                        c = 2 * b + s
                        nc.scalar.activation(
                            out=P[:, c * GS:(c + 1) * GS],
                            in_=S_b[:, s * GS:s * GS + GS],
                            func=mybir.ActivationFunctionType.Exp,
                            bias=negm_b[:, s:s + 1], scale=1.0,
                        )
                # 8. P^T per head
                PT_ps = ps_pt.tile([128, NH * GS], fp16, tag="PT_ps")
                for c in range(NH):
                    nc.tensor.transpose(
                        out=PT_ps[:, c * GS:(c + 1) * GS],
                        in_=P[:, c * GS:(c + 1) * GS],
                        identity=ident_16[:],
                    )
                PT = sb.tile([128, NH * GS], fp16, tag="PT")
                nc.vector.tensor_copy(out=PT[:, 0:384], in_=PT_ps[:, 0:384])
                nc.scalar.copy(out=PT[:, 384:768], in_=PT_ps[:, 384:768])

                # 9. [O_h | D_h] per head in one matmul (rhs = [v_h | 1])
                OD_ps = ps_b.tile([128, NH * (D + 1)], f32, tag="odr")
                for c in range(NH):
                    h = PERM[c]
                    nc.tensor.matmul(
                        out=OD_ps[:, h * (D + 1):(h + 1) * (D + 1)],
                        lhsT=PT[:, c * GS:(c + 1) * GS],
                        rhs=v_bf[:, h * (D + 1):(h + 1) * (D + 1)],
                        start=(c == 0), stop=(c == NH - 1), skip_group_check=True,
                    )
                # 10. divide (strided D cols at offset 32 of each 33-block)
                recip = sb.tile([128, NH], f32, tag="recip")
                nc.vector.reciprocal(
                    out=recip[:],
                    in_=OD_ps[:].rearrange("p (h e) -> p h e", h=NH)[:, :, D:D + 1],
                )
                Obar = sb.tile([128, C + 2], fp16, tag="Obar")
                nc.vector.tensor_tensor(
                    out=Obar[:, 0:C].rearrange("p (h d) -> p h d", h=NH),
                    in0=OD_ps[:].rearrange("p (h e) -> p h e", h=NH)[:, :, 0:D],
                    in1=recip[:, :, None].to_broadcast([128, NH, D]),
                    op=mybir.AluOpType.mult,
                )
                nc.vector.memset(Obar[:, C:C + 1], 1.0)
                # 11. ObarT
                ObarT_ps = ps_b.tile([97, 256], fp16, tag="odr")
                nc.tensor.transpose(
                    out=ObarT_ps[0:96, 0:128], in_=Obar[:, 0:96], identity=ident_16[:]
                )
                nc.tensor.transpose(
                    out=ObarT_ps[0:97, 128:256], in_=Obar[:, 96:193], identity=ident_16[:]
                )
                ObarT = sb.tile([97, 256], fp16, tag="ObarT")
                nc.vector.tensor_copy(out=ObarT[:], in_=ObarT_ps[:])
                # 12. projection (bias folded into wt1 row 96 / ObarT ones row)
                R_ps = ps_b.tile([128, C], f32, tag="odr")
                nc.tensor.matmul(out=R_ps[:], lhsT=ObarT[0:96, 0:128], rhs=wt0[:],
                                 start=True, stop=False, skip_group_check=True)
                nc.tensor.matmul(out=R_ps[:], lhsT=ObarT[0:97, 128:256], rhs=wt1[:],
                                 start=False, stop=True, skip_group_check=True)
                R = gp.tile([128, C], f32, tag="R")
                nc.vector.tensor_copy(out=R[:], in_=R_ps[:])
                # 13. scatter
                nc.gpsimd.indirect_dma_start(
                    out=out_d[:],
                    out_offset=bass.IndirectOffsetOnAxis(
                        ap=idx_all[:, g:g + 1], axis=0
                    ),
                    in_=R[:],
                    in_offset=None,
                )

    _fixup_multiwait(nc)
    return nc


_NC_CACHE = {}


def _get_nc():
    if "nc" not in _NC_CACHE:
        _patch_tile()
        _install_ntff_hook()
        _NC_CACHE["nc"] = build_nc()
    return _NC_CACHE["nc"]


def _host_prep(qkv, indices, proj_w, proj_b, logit_scale):
    qkv = np.asarray(qkv, dtype=np.float32)
    idx = np.asarray(indices)
    proj_w = np.asarray(proj_w, dtype=np.float32)
    proj_b = np.asarray(proj_b, dtype=np.float32)
    lsc = float(np.asarray(logit_scale).reshape(-1)[0])

    tk = idx.reshape(B, N)
    order = np.argsort(tk, axis=-1, kind="stable").astype(np.int32)  # (B, N)

    wt = np.ascontiguousarray(
        np.concatenate([proj_w.T, proj_b[None, :]], axis=0)
    )
    lsc_vec = np.full((128, 1), lsc, np.float32)

    in_maps = []
    for k in range(N_CORES):
        b = k // 2
        half = k % 2
        cols = order[b, half * 8192:(half + 1) * 8192].reshape(NG_CORE, 128).T
        in_maps.append(
            {
                "qkvb": np.ascontiguousarray(qkv[b]),
                "ordv": np.ascontiguousarray(cols),
                "wt": wt,
                "lsc": lsc_vec,
            }
        )
    return in_maps


def kernel(qkv, indices, proj_w, proj_b, logit_scale, _trace=False, _tmpdir=None):
    nc = _get_nc()
    in_maps = _host_prep(qkv, indices, proj_w, proj_b, logit_scale)
    res = run_bass_kernel_spmd(
        nc, in_maps, list(range(N_CORES)), trace=_trace, tmpdir=_tmpdir
    )
    out = np.empty((B, N, C), np.float32)
    for b in range(B):
        out[b] = res.results[2 * b]["out"] + res.results[2 * b + 1]["out"]
    kernel.last_exec_time_ns = res.exec_time_ns
    return out
